# revision 9
# baseline (speedup 1.0000x reference)
"""CrossModalAttention TRN2 kernel v4: PE does all reductions.

Measured on v3: GpSimd shares an SBUF port with the DVE — concurrent
gpsimd ops halve DVE throughput (slow DVE slices overlap gpsimd 95%), so
gpsimd offload is strictly counterproductive. v4 removes gpsimd from the
datapath and moves every reduction onto the (otherwise idle) TensorE via
identity-weight accumulating matmuls:

 - QK: DVE computes the 9 neighbor products (bf16, 2x); PE accumulates
   the 16 token pixels with 16 N=128 matmuls per neighbor into a PSUM
   score tile (fp32). ACT's exp reads the scores straight from PSUM.
 - AV: DVE computes the 9 weighted products; PE accumulates them with
   4 N=512 matmuls per neighbor into a PSUM tile; ACT evacuates with the
   pixel-order permute, bf16 out.
 - Projections all upfront (weight-stationary blocks, evacuations split
   ACT/DVE); their PSUM pools close before the attention PSUM pools open
   so the 8 banks are never oversubscribed.
 - Normalize-first softmax as in v3 (a = (e*mask)*(1/Z) at token
   resolution, expanded by broadcast DMA).
"""

import os
from contextlib import ExitStack

import numpy as np

import concourse.bass as bass
import concourse.mybir as mybir
import concourse.tile as tile
from concourse.bass_utils import run_bass_kernel_spmd

B, C, H, W = 2, 256, 128, 128
TOK = 4
NH, NW = H // TOK, W // TOK
T2 = TOK * TOK
SCALE = float((C // T2) ** -0.5)
NCORES = 8
QH = 4
NH_LOC = NH // QH                    # 8 token rows / core
PIX_LOC = NH_LOC * TOK * W           # 4096
HALO_ROWS = NH_LOC + 2
PIX_HALO = HALO_ROWS * TOK * W       # 5120
ROWSZ = TOK * W                      # 512
G = 4                                # token rows per attention group
NG = NH_LOC // G                     # 2 groups
GSZ = G * ROWSZ                      # 2048 pixels per group
GNJ = G * NW                         # 128 token positions per group
OFFS = [(di, dj) for di in (-1, 0, 1) for dj in (-1, 0, 1)]
NN = len(OFFS)

F32 = mybir.dt.float32
FP16 = mybir.dt.float16
BF16 = mybir.dt.bfloat16
AX = mybir.AxisListType
AF = mybir.ActivationFunctionType
ALU = mybir.AluOpType


def _build_kernel(nc: bass.Bass, ctx: ExitStack, tc: "tile.TileContext"):
    xb = nc.dram_tensor("xb", [C, PIX_LOC], FP16, kind="ExternalInput").ap()
    xw = nc.dram_tensor("xw", [C, PIX_HALO], FP16, kind="ExternalInput").ap()
    wq = nc.dram_tensor("wq", [C, C], FP16, kind="ExternalInput").ap()
    wk = nc.dram_tensor("wk", [C, C], FP16, kind="ExternalInput").ap()
    wv = nc.dram_tensor("wv", [C, C], FP16, kind="ExternalInput").ap()
    bq = nc.dram_tensor("bq", [2, 128, 1], F32, kind="ExternalInput").ap()
    bk = nc.dram_tensor("bk", [2, 128, 1], F32, kind="ExternalInput").ap()
    bv = nc.dram_tensor("bv", [2, 128, 1], F32, kind="ExternalInput").ap()
    ident = nc.dram_tensor("ident", [128, 128], BF16,
                           kind="ExternalInput").ap()
    mask = nc.dram_tensor("mask", [128, NH_LOC * NW * NN], BF16,
                          kind="ExternalInput").ap()
    out = nc.dram_tensor("out", [C, PIX_LOC], BF16, kind="ExternalOutput").ap()

    const_pool = ctx.enter_context(tc.tile_pool(name="const", bufs=1))
    qkv_pool = ctx.enter_context(tc.tile_pool(name="qkv", bufs=1))
    # ---- persistent constants
    w_sb = {}
    for name, wd in (("q", wq), ("k", wk), ("v", wv)):
        for ci in range(2):
            t = const_pool.tile([128, C], FP16, tag=f"w{name}{ci}",
                                name=f"w{name}{ci}")
            nc.sync.dma_start(t[:], wd[ci * 128:(ci + 1) * 128, :])
            w_sb[name, ci] = t
    b_sb = {}
    for name, bd in (("q", bq), ("k", bk), ("v", bv)):
        for co in range(2):
            t = const_pool.tile([128, 1], F32, tag=f"b{name}{co}",
                                name=f"b{name}{co}")
            nc.sync.dma_start(t[:], bd[co])
            b_sb[name, co] = t
    id_sb = const_pool.tile([128, 128], BF16, tag="ident", name="ident")
    nc.sync.dma_start(id_sb[:], ident[:])
    mask_sb = const_pool.tile([128, NH_LOC * NW * NN], BF16, tag="mask")
    nc.sync.dma_start(mask_sb[:], mask[:])

    # ACT/DVE warm-ups: cover the bias/mask DMAs on their consumer engines
    # (walrus 1-sync-wait limit) before the real consumers run. The dummy
    # Exp also pulls in the ACT exp table load during the projection phase.
    scratch = const_pool.tile([128, 8], F32, tag="scratch", name="scratch")
    for wi, name in enumerate(("q", "k", "v")):
        for co in range(2):
            nc.scalar.activation(scratch[:, wi * 2 + co:wi * 2 + co + 1],
                                 b_sb[name, co][:], AF.Identity,
                                 bias=b_sb[name, co][:])
            nc.vector.tensor_scalar(scratch[:, wi * 2 + co:wi * 2 + co + 1],
                                    b_sb[name, co][:], b_sb[name, co][:],
                                    None, ALU.add)
    nc.vector.tensor_copy(scratch[:, 6:7], mask_sb[:, 0:1])
    nc.scalar.activation(scratch[:, 7:8], scratch[:, 6:7], AF.Exp)

    q_sb = [qkv_pool.tile([128, PIX_LOC], BF16, tag=f"q{c}", name=f"q{c}")
            for c in range(2)]
    k_sb = [qkv_pool.tile([128, PIX_HALO + 2 * TOK], BF16, tag=f"k{c}",
                          name=f"k{c}") for c in range(2)]
    v_sb = [qkv_pool.tile([128, PIX_HALO + 2 * TOK], BF16, tag=f"v{c}",
                          name=f"v{c}") for c in range(2)]
    for t in (*k_sb, *v_sb):
        nc.vector.memset(t[:, 0:TOK], 0.0)
        nc.vector.memset(t[:, TOK + PIX_HALO:], 0.0)

    x_pool = ctx.enter_context(tc.tile_pool(name="x", bufs=1))
    evac_flip = [0]

    # ---- projections (own PSUM scope, closed before attention PSUM opens)
    with tc.tile_pool(name="ps", bufs=7, space="PSUM") as ps_pool, \
         tc.tile_pool(name="warmps", bufs=1, space="PSUM") as warm_pool:
        warm_ps = warm_pool.tile([128, 8], F32, tag="warm")
        for name in ("q", "k", "v"):
            for ci in range(2):
                nc.tensor.matmul(warm_ps[0:1, 0:1], w_sb[name, ci][:, 0:1],
                                 w_sb[name, ci][:, 0:1],
                                 start=True, stop=True)
        nc.tensor.matmul(warm_ps[0:1, 0:1], id_sb[:, 0:1], id_sb[:, 0:1],
                         start=True, stop=True)

        # x inputs: one-shot full buffers (no WAW waits on DMA triggers);
        # dummy matmuls per 512-slice put every x-DMA queue on PE's clock.
        xb_sb = [x_pool.tile([128, PIX_LOC], FP16, tag=f"xb{ci}",
                             name=f"xb{ci}") for ci in range(2)]
        xw_sb = [x_pool.tile([128, PIX_HALO], FP16, tag=f"xw{ci}",
                             name=f"xw{ci}") for ci in range(2)]
        for ci in range(2):
            rows = slice(ci * 128, (ci + 1) * 128)
            for c0 in range(0, PIX_HALO, 1024):
                nc.sync.dma_start(xw_sb[ci][:, c0:c0 + 1024],
                                  xw[rows, c0:c0 + 1024])
            for c0 in range(0, PIX_LOC, 1024):
                nc.sync.dma_start(xb_sb[ci][:, c0:c0 + 1024],
                                  xb[rows, c0:c0 + 1024])
            for pt in range(PIX_HALO // 512):
                nc.tensor.matmul(warm_ps[0:1, 0:1],
                                 xw_sb[ci][:, pt * 512:pt * 512 + 1],
                                 xw_sb[ci][:, pt * 512:pt * 512 + 1],
                                 start=True, stop=True)
            for pt in range(PIX_LOC // 512):
                nc.tensor.matmul(warm_ps[0:1, 0:1],
                                 xb_sb[ci][:, pt * 512:pt * 512 + 1],
                                 xb_sb[ci][:, pt * 512:pt * 512 + 1],
                                 start=True, stop=True)

        def project_block(name, x_src, dst, scale, pad, pts):
            for co in range(2):
                co_sl = slice(co * 128, (co + 1) * 128)
                ps_ts = []
                for ci in range(2):
                    for i, pt in enumerate(pts):
                        sl = slice(pt * 512, (pt + 1) * 512)
                        if name == "v":
                            xs = x_src[ci][:, sl].rearrange(
                                "p (u j v) -> p u v j", u=TOK, j=NW, v=TOK)
                        else:
                            xs = x_src[ci][:, sl]
                        if ci == 0:
                            ps = ps_pool.tile([128, 512], F32, tag="pp")
                            ps_ts.append(ps)
                            nc.tensor.matmul(ps[:], w_sb[name, 0][:, co_sl],
                                             xs, start=True, stop=False)
                        else:
                            nc.tensor.matmul(ps_ts[i][:],
                                             w_sb[name, 1][:, co_sl],
                                             xs, start=False, stop=True)
                for i, pt in enumerate(pts):
                    dsl = slice(pad + pt * 512, pad + (pt + 1) * 512)
                    ps = ps_ts[i]
                    if evac_flip[0] % 2 == 0:
                        nc.scalar.activation(dst[co][:, dsl], ps[:],
                                             AF.Identity,
                                             bias=b_sb[name, co][:],
                                             scale=scale)
                    elif scale == 1.0:
                        nc.vector.tensor_scalar(dst[co][:, dsl], ps[:],
                                                b_sb[name, co][:], None,
                                                ALU.add)
                    else:
                        nc.vector.tensor_scalar(dst[co][:, dsl], ps[:],
                                                scale, b_sb[name, co][:],
                                                ALU.mult, ALU.add)
                    evac_flip[0] += 1

        # K first (QK products need it), Q second, V last (AV is later)
        project_block("k", xw_sb, k_sb, 1.0, TOK, list(range(0, 7)))
        project_block("k", xw_sb, k_sb, 1.0, TOK, list(range(7, 10)))
        project_block("q", xb_sb, q_sb, SCALE, 0, list(range(0, 4)))
        project_block("q", xb_sb, q_sb, SCALE, 0, list(range(4, 8)))
        project_block("v", xw_sb, v_sb, 1.0, TOK, list(range(0, 7)))
        project_block("v", xw_sb, v_sb, 1.0, TOK, list(range(7, 10)))

    # ---- attention pools
    prod_pool = ctx.enter_context(tc.tile_pool(name="prod", bufs=4))
    e_pool = ctx.enter_context(tc.tile_pool(name="e", bufs=2))
    z_pool = ctx.enter_context(tc.tile_pool(name="z", bufs=2))
    ax_pool = ctx.enter_context(tc.tile_pool(name="ax", bufs=6))
    avp_pool = ctx.enter_context(tc.tile_pool(name="avp", bufs=3))
    on_pool = ctx.enter_context(tc.tile_pool(name="on", bufs=2))
    sps_pool = ctx.enter_context(tc.tile_pool(name="sps", bufs=1,
                                              space="PSUM"))
    ops_pool = ctx.enter_context(tc.tile_pool(name="ops", bufs=1,
                                              space="PSUM"))

    state = {}

    def qk_phase(ch, g):
        qsl = q_sb[ch][:, g * GSZ:(g + 1) * GSZ]
        s_ps = sps_pool.tile([128, NN * GNJ], F32, tag="sps", name="sps")
        for n, (di, dj) in enumerate(OFFS):
            koff = TOK + (g * G + 1 + di) * ROWSZ + dj * TOK
            prod = prod_pool.tile([128, GSZ], BF16, tag="prod", name="prod")
            nc.vector.tensor_mul(prod[:], qsl, k_sb[ch][:, koff:koff + GSZ])
            # PE reduces the 16 token pixels: 16 accumulating N=128
            # matmuls against the identity (rhs strided per (u,v))
            pview = prod[:].rearrange("p (i u j v) -> p u v i j",
                                      i=G, u=TOK, j=NW, v=TOK)
            for t in range(T2):
                u, v = divmod(t, TOK)
                nc.tensor.matmul(s_ps[:, n * GNJ:(n + 1) * GNJ],
                                 id_sb[:], pview[:, u, v],
                                 start=(t == 0), stop=(t == T2 - 1))
        e_t = e_pool.tile([128, NN * GNJ], BF16, tag="e", name="e")
        nc.scalar.activation(e_t[:], s_ps[:], AF.Exp)
        state[ch, g, "e"] = e_t

    def softmax_phase(ch, g):
        e_t = state.pop((ch, g, "e"))
        em_t = e_pool.tile([128, NN * GNJ], BF16, tag="em", name="em")
        msl = mask_sb[:, g * NN * GNJ:(g + 1) * NN * GNJ]
        nc.vector.tensor_mul(em_t[:], e_t[:], msl)
        # Z: bf16 tree over the 9 neighbor stripes, fp32 final stage
        t1 = z_pool.tile([128, 4 * GNJ], BF16, tag="t1", name="t1")
        nc.vector.tensor_add(t1[:], em_t[:, 0:4 * GNJ],
                             em_t[:, 4 * GNJ:8 * GNJ])
        t2 = z_pool.tile([128, 2 * GNJ], BF16, tag="t2", name="t2")
        nc.vector.tensor_add(t2[:], t1[:, 0:2 * GNJ], t1[:, 2 * GNJ:4 * GNJ])
        t3 = z_pool.tile([128, GNJ], BF16, tag="t3", name="t3")
        nc.vector.tensor_add(t3[:], t2[:, 0:GNJ], t2[:, GNJ:2 * GNJ])
        z_t = z_pool.tile([128, GNJ], F32, tag="z", name="z")
        nc.vector.tensor_add(z_t[:], t3[:], em_t[:, 8 * GNJ:9 * GNJ])
        zr_t = z_pool.tile([128, GNJ], F32, tag="zr", name="zr")
        nc.vector.reciprocal(zr_t[:], z_t[:])
        # normalized weights at token resolution
        a_t = e_pool.tile([128, NN * GNJ], BF16, tag="a", name="a")
        nc.vector.tensor_mul(
            a_t[:].rearrange("p (n ij) -> p n ij", n=NN, ij=GNJ),
            em_t[:].rearrange("p (n ij) -> p n ij", n=NN, ij=GNJ),
            zr_t[:].unsqueeze(1).broadcast_to((128, NN, GNJ)))
        # expand each neighbor's weights over the 16 token pixels (DMA,
        # step-0 src dims)
        ax_ts = {}
        for n in range(NN):
            a_x = ax_pool.tile([128, GSZ], BF16, tag="ax", name="ax")
            asl = a_t[:, n * GNJ:(n + 1) * GNJ]
            nc.sync.dma_start(
                a_x[:].rearrange("p (r f) -> p r f", r=T2, f=GNJ),
                asl.unsqueeze(1).broadcast_to((128, T2, GNJ)))
            ax_ts[n] = a_x
        state[ch, g, "ax"] = ax_ts

    def av_phase(ch, g):
        ax_ts = state.pop((ch, g, "ax"))
        o_ps = ops_pool.tile([128, GSZ], F32, tag="ops", name="ops")
        for n, (di, dj) in enumerate(OFFS):
            voff = TOK + (g * G + 1 + di) * ROWSZ + dj
            vv = v_sb[ch][:, voff:voff + GSZ].rearrange(
                "p (i uv j) -> p i uv j", i=G, uv=T2, j=NW)
            tmp = avp_pool.tile([128, GSZ], BF16, tag="avt", name="avt")
            tv = tmp[:].rearrange("p (i uv j) -> p i uv j",
                                  i=G, uv=T2, j=NW)
            axv = ax_ts[n][:].rearrange("p (uv i j) -> p i uv j",
                                        uv=T2, i=G, j=NW)
            nc.vector.tensor_mul(tv, axv, vv)
            # PE accumulates the 9 neighbor products into PSUM
            for c0 in range(0, GSZ, 512):
                nc.tensor.matmul(o_ps[:, c0:c0 + 512], id_sb[:],
                                 tmp[:, c0:c0 + 512],
                                 start=(n == 0), stop=(n == NN - 1))
        # permute back to pixel order on ACT, store bf16
        on = on_pool.tile([128, GSZ], BF16, tag="on", name="on")
        onv = on[:].rearrange("p (iu j v) -> p iu j v",
                              iu=G * TOK, j=NW, v=TOK)
        ov = o_ps[:].rearrange("p (iu v j) -> p iu j v",
                               iu=G * TOK, v=TOK, j=NW)
        nc.scalar.copy(onv, ov)
        nc.sync.dma_start(
            out[ch * 128:(ch + 1) * 128, g * GSZ:(g + 1) * GSZ], on[:])

    # ---- pipelined attention emission
    g1, g2, g3, g4 = (0, 0), (1, 0), (0, 1), (1, 1)   # (ch, g), g-major
    qk_phase(*g1)
    qk_phase(*g2)
    softmax_phase(*g1)
    qk_phase(*g3)
    softmax_phase(*g2)
    av_phase(*g1)
    qk_phase(*g4)
    softmax_phase(*g3)
    av_phase(*g2)
    softmax_phase(*g4)
    av_phase(*g3)
    av_phase(*g4)


_CACHE = {}


def _ldw_key(ins):
    return (str(ins.ins[0]), str(ins.is_transpose), str(ins.tile_position),
            str(ins.perf_mode))


def _dedupe_ldweights(nc):
    """Drop Ldweights that reload the identical stationary operand.

    bass emits one Ldweights per matmul; runs of identity-weight matmuls
    (the PE reduction trick) reload the same 128x128 for every matmul,
    serializing ~110ns each on PE. A redundant Ldweights is replaced by a
    sync-only EventSemaphore when it carries sync commands, else removed.
    """
    f = nc.m.functions[0]
    for blk in f.blocks:
        il = blk.instructions
        last = None
        i = 0
        while i < len(il):
            ins = il[i]
            if ins.opcode == "Ldweights":
                key = _ldw_key(ins)
                if key == last:
                    si = ins.sync_info
                    has_sync = si is not None and (
                        len(list(si.on_wait)) or len(list(si.on_update)))
                    if has_sync:
                        ev = mybir.InstEventSemaphore(
                            name=f"ldw-{nc.next_id()}", ins=[], outs=[])
                        ev.engine = ins.engine
                        ev.sync_info = si
                        il[i] = ev
                    else:
                        del il[i]
                        continue
                else:
                    last = key
            i += 1


# --- post-scheduling legalization: move excess sync waits/updates onto
# standalone EventSemaphore instructions (walrus ISA slot limits).
WAIT_LIMIT = 1
UPDATE_LIMIT = 1


def _legalize_waits(nc):
    f = nc.m.functions[0]
    for blk in f.blocks:
        il = blk.instructions
        i = 0
        while i < len(il):
            ins = il[i]
            si = ins.sync_info
            if si is None or ins.opcode == "EventSemaphore":
                i += 1
                continue
            waits = list(si.on_wait)
            ups = list(si.on_update)
            changed = False
            if len(waits) > WAIT_LIMIT:
                excess, waits = waits[:-WAIT_LIMIT], waits[-WAIT_LIMIT:]
                for w in excess:
                    ev = mybir.InstEventSemaphore(
                        name=f"lgw-{nc.next_id()}", ins=[], outs=[])
                    ev.engine = ins.engine
                    ev.sync_info = mybir.SyncInfo(on_wait=[w], on_update=[])
                    il.insert(i, ev)
                    i += 1
            post = []
            if len(ups) > UPDATE_LIMIT:
                excess_u, ups = ups[UPDATE_LIMIT:], ups[:UPDATE_LIMIT]
                for u in excess_u:
                    ev = mybir.InstEventSemaphore(
                        name=f"lgu-{nc.next_id()}", ins=[], outs=[])
                    ev.engine = ins.engine
                    ev.sync_info = mybir.SyncInfo(on_wait=[], on_update=[u])
                    post.append(ev)
                changed = True
            if changed or len(list(si.on_wait)) > WAIT_LIMIT:
                ins.sync_info = mybir.SyncInfo(on_wait=waits, on_update=ups)
            for ev in post:
                i += 1
                il.insert(i, ev)
            i += 1


def _get_program():
    if "nc" not in _CACHE:
        nc = bass.Bass("TRN2", target_bir_lowering=False, debug=False)
        with tile.TileContext(nc) as tc:
            with ExitStack() as ctx:
                _build_kernel(nc, ctx, tc)
        if os.environ.get("KERNEL_NO_DEDUP") != "1":
            _dedupe_ldweights(nc)
        if os.environ.get("KERNEL_NO_LEGALIZE") != "1":
            _legalize_waits(nc)
        _CACHE["nc"] = nc
    return _CACHE["nc"]


def _shard_inputs(blue_feat, white_feat, q_w, q_b, k_w, k_b, v_w, v_b):
    import ml_dtypes
    blue = np.ascontiguousarray(blue_feat, dtype=np.float16)
    white = np.ascontiguousarray(white_feat, dtype=np.float16)
    wts = {
        "wq": np.ascontiguousarray(np.asarray(q_w, np.float16).T),
        "wk": np.ascontiguousarray(np.asarray(k_w, np.float16).T),
        "wv": np.ascontiguousarray(np.asarray(v_w, np.float16).T),
        "bq": (np.asarray(q_b, np.float32) * SCALE).reshape(2, 128, 1).copy(),
        "bk": np.asarray(k_b, np.float32).reshape(2, 128, 1).copy(),
        "bv": np.asarray(v_b, np.float32).reshape(2, 128, 1).copy(),
        "ident": np.eye(128, dtype=ml_dtypes.bfloat16),
    }
    in_maps = []
    for core in range(NCORES):
        b, q = divmod(core, QH)
        r0 = q * NH_LOC * TOK
        xb_ = blue[b, :, r0:r0 + NH_LOC * TOK, :].reshape(C, PIX_LOC)
        xw_ = np.zeros((C, HALO_ROWS * TOK, W), np.float16)
        lo, hi = r0 - TOK, r0 + (NH_LOC + 1) * TOK
        slo, shi = max(lo, 0), min(hi, H)
        xw_[:, slo - lo:shi - lo, :] = white[b, :, slo:shi, :]
        xw_ = xw_.reshape(C, PIX_HALO)
        gi = q * NH_LOC + np.arange(NH_LOC)[:, None, None]
        j = np.arange(NW)[None, :, None]
        di = np.array([o[0] for o in OFFS])[None, None, :]
        dj = np.array([o[1] for o in OFFS])[None, None, :]
        m = ((gi + di >= 0) & (gi + di < NH) &
             (j + dj >= 0) & (j + dj < NW)).astype(np.float32)
        # [i, j, n] -> [g, n, i_in_group, j] to match the n-outer score
        # layout
        m = m.reshape(NG, G, NW, NN).transpose(0, 3, 1, 2).reshape(-1)
        m = np.broadcast_to(m.reshape(1, -1), (128, NH_LOC * NW * NN))
        m = m.astype(ml_dtypes.bfloat16).copy()
        in_maps.append({"xb": np.ascontiguousarray(xb_),
                        "xw": np.ascontiguousarray(xw_),
                        "mask": m, **wts})
    return in_maps


def _assemble(results):
    out = np.empty((B, C, H, W), np.float32)
    for core in range(NCORES):
        b, q = divmod(core, QH)
        r0 = q * NH_LOC * TOK
        out[b, :, r0:r0 + NH_LOC * TOK, :] = \
            results[core]["out"].astype(np.float32).reshape(
                C, NH_LOC * TOK, W)
    return out


def kernel(blue_feat, white_feat, q_w, q_b, k_w, k_b, v_w, v_b):
    nc = _get_program()
    in_maps = _shard_inputs(blue_feat, white_feat,
                            q_w, q_b, k_w, k_b, v_w, v_b)
    trace = os.environ.get("KERNEL_TRACE") == "1"
    res = run_bass_kernel_spmd(nc, in_maps, core_ids=list(range(NCORES)),
                               trace=trace)
    if trace:
        _CACHE["last_result"] = res
    return _assemble(res.results)
